# revision 28
# baseline (speedup 1.0000x reference)
"""Trainium2 Bass kernel for BART custom-mask attention.

Problem: B=4, T=S=1024, E=1024, H=16 heads, D=64.
  q = (hs @ q_w.T + q_b) * D**-0.5 ; k/v analogous
  scores = q k^T + attention_mask ; attn = softmax(scores)
  attn(head h) *= (1-hm[h]) + hm[h]*(relation_inputs>0)   (no renorm)
  out = (attn @ v) @ o_w.T + o_b

Sharding: 8 cores = batch (4) x head-group (2, 8 heads each). Each core
computes a 512-feature slice of the attention output and projects it
through the matching o_w columns; the host sums the two half-partials
per batch (plus o_b, folded into the host gather).

Per-core design (bf16 matmuls for projections/scores, fp8e4 for the
exp/V side, fp32 PSUM):

  - The ScalarE exp stream (64 [128,1024]-tile activations, ~75us) is
    the pacing engine. The emission schedule issues the 4 score matmuls
    of one (pair, sc) step, the single [128,2048] exp, then ~2 "filler"
    units of other PE work (qk / v projections, denominator matmuls,
    attn@v) so the PE stays busy exactly while ACT drains the previous
    score tile. PSUM: one [128,2048] score tile (4 banks, bufs=1) + a
    [128,1024] work ring (bufs=2, 4 banks).
  - Score matmuls are K=64 row-tiled pairs (head A on partitions 0:63,
    head B on 64:127) issued back-to-back so the PE runs them
    concurrently in the two halves of the array.
  - exp writes fp8e4 E tiles [128, 2048] = [eA-th0|eA-th1|eB-th0|eB-th1].
  - attn@v: lhsT = fp8 [ones(64) | v_h] 128-col blocks from one mega
    tile; PSUM rows 0:63 get the softmax denominator, 64:127 the data.
  - relation-masked slots: unmasked denominator via a ones128 matmul,
    reciprocal stashed to SBUF, then E *= relM in place on GpSimd.
  - normalize: reciprocal_approx_fast at partition base 0, SBUF->SBUF
    DMA re-homes to partitions 64:127, one DVE mul per head.
  - output projection contracts the 512 features, ScalarE evacuates
    (free after the exp stream), y written bf16.
"""

import os
import sys

import numpy as np

for _p in ("/opt/trn_rl_repo", "/root/.axon_site/_ro/trn_rl_repo"):
    if os.path.isdir(_p) and _p not in sys.path:
        sys.path.insert(0, _p)
        break

import ml_dtypes

B, T, E, H = 4, 1024, 1024, 16
D = E // H
SCALING = D ** -0.5
N_CORES = 8
FH = 512          # features per core (8 heads x 64)
P = 128
BF16 = ml_dtypes.bfloat16
FP8 = ml_dtypes.float8_e4m3

_PROGS = {}


def _build_program(mask_on, slot_flags):
    import concourse.tile as tile
    from concourse import bacc, mybir
    from contextlib import ExitStack

    bf = mybir.dt.bfloat16
    f32 = mybir.dt.float32
    f8 = mybir.dt.float8e4
    Exp = mybir.ActivationFunctionType.Exp

    nc = bacc.Bacc("TRN2", target_bir_lowering=False, debug=False,
                   num_devices=N_CORES)

    xT_d = nc.declare_dram_parameter("xT", [E, T], bf, isOutput=False)
    wqT_d = nc.declare_dram_parameter("wqT", [E, FH], bf, isOutput=False)
    wkT_d = nc.declare_dram_parameter("wkT", [E, FH], bf, isOutput=False)
    wvT_d = nc.declare_dram_parameter("wvT", [E, FH], bf, isOutput=False)
    owT_d = nc.declare_dram_parameter("owT", [FH, E], bf, isOutput=False)
    qb_d = nc.declare_dram_parameter("qb", [P, 4], f32, isOutput=False)
    kb_d = nc.declare_dram_parameter("kb", [P, 4], f32, isOutput=False)
    vbb_d = nc.declare_dram_parameter("vbb", [P, FH], f32, isOutput=False)
    relM_d = {}
    for k in range(8):
        if slot_flags[k]:
            relM_d[k] = nc.declare_dram_parameter(f"relM{k}", [T, T], f8,
                                                  isOutput=False)
    if mask_on:
        expm_d = nc.declare_dram_parameter("expmaskT", [T, T], bf,
                                           isOutput=False)
    y_d = nc.declare_dram_parameter("y", [T, E], bf, isOutput=True)

    with tile.TileContext(nc) as tc, ExitStack() as ctx:
        persist = ctx.enter_context(tc.tile_pool(name="persist", bufs=1))

        ones128 = persist.tile([P, P], bf, name="ones128", tag="ones128")
        nc.vector.memset(ones128[:], 1.0)
        warm_t = persist.tile([P, 512], bf, name="warm_t", tag="warm_t")
        nc.vector.memset(warm_t[:], 0.5)
        mega = persist.tile([P, 8192], bf, name="vmega", tag="vmega")
        nc.vector.memset(
            mega[:].rearrange("p (g c) -> p g c", c=128)[:, :, 0:64], 1.0)

        # ---------------- input DMA, spread over HWDGE queues ----------
        crit_engines = [nc.sync, nc.scalar]
        late_engines = [nc.gpsimd, nc.sync, nc.scalar, nc.gpsimd]
        dma_rr = [0]

        def dma_in(out_ap, in_ap, late=False):
            engs = late_engines if late else crit_engines
            eng = engs[dma_rr[0] % len(engs)]
            dma_rr[0] += 1
            eng.dma_start(out_ap, in_ap)

        def alloc_tiles(n, rows, cols, nm, dt=bf):
            return [persist.tile([rows, cols], dt, name=f"{nm}{i}",
                                 tag=f"{nm}{i}") for i in range(n)]

        xT_t = alloc_tiles(8, P, T, "xTt")
        wqT_t = alloc_tiles(8, P, FH, "wqTt")
        wkT_t = alloc_tiles(8, P, FH, "wkTt")
        wvT_t = alloc_tiles(8, P, FH, "wvTt")
        owT_t = alloc_tiles(4, P, T, "owTt")
        relM_t = {k: alloc_tiles(8, P, T, f"rMt{k}", dt=f8) for k in relM_d}
        if mask_on:
            expm_t = alloc_tiles(8, P, T, "emt")

        qb_t = persist.tile([P, 4], f32, name="qbt", tag="qbt")
        kb_t = persist.tile([P, 4], f32, name="kbt", tag="kbt")
        vbb_t = persist.tile([P, FH], f32, name="vbbt", tag="vbbt")

        dma_in(qb_t[:], qb_d[:])
        dma_in(kb_t[:], kb_d[:])
        dma_in(vbb_t[:], vbb_d[:])
        # pair-0-critical first: xT + wq/wk chunk by chunk over the two
        # HWDGE queues
        for ec in range(8):
            dma_in(xT_t[ec][:], xT_d[P * ec:P * (ec + 1), :])
            dma_in(wqT_t[ec][:], wqT_d[P * ec:P * (ec + 1), :])
            dma_in(wkT_t[ec][:], wkT_d[P * ec:P * (ec + 1), :])
        for ec in range(8):
            dma_in(wvT_t[ec][:], wvT_d[P * ec:P * (ec + 1), :])
        dma_rr[0] = 0
        for k, dparm in relM_d.items():
            for i in range(8):
                dma_in(relM_t[k][i][:], dparm[P * i:P * (i + 1), :],
                       late=True)
        if mask_on:
            for i in range(8):
                dma_in(expm_t[i][:], expm_d[P * i:P * (i + 1), :],
                       late=True)
        for i in range(4):
            dma_in(owT_t[i][:], owT_d[P * i:P * (i + 1), :], late=True)

        # ---------------- persistent SBUF working tiles ----------------
        qT_t = [persist.tile([P, T], bf, name=f"qTs{p}", tag=f"qTs{p}")
                for p in range(4)]
        kT_t = [persist.tile([P, T], bf, name=f"kTs{p}", tag=f"kTs{p}")
                for p in range(4)]
        # v mega tile: col = lh*1024 + s*128 + c; c in 0:64 -> ones
        # (denominator trick), 64:128 -> v_h for s-chunk s.
        oT_sb = [persist.tile([P, T], bf, name=f"oTs{p}", tag=f"oTs{p}")
                 for p in range(4)]
        du_recip = {}
        for lh in range(8):
            if slot_flags[lh]:
                du_recip[lh] = persist.tile([64, T], f32, name=f"dur{lh}",
                                            tag=f"dur{lh}")

        e_t = {}          # (p, sc) -> fp8 [128, 2048] E tile
        av_ps = {}        # lh -> [128, 1024] f32 PSUM tile

        e_pool = ctx.enter_context(tc.tile_pool(name="e_sb", bufs=1))
        cb_pool = ctx.enter_context(tc.tile_pool(name="cb_sb", bufs=1))
        ysb_pool = ctx.enter_context(tc.tile_pool(name="y_sb", bufs=1))
        w_pool_cm = tc.tile_pool(name="w_ps", bufs=1, space="PSUM")
        w_pool = w_pool_cm.__enter__()
        s_pool_cm = tc.tile_pool(name="s_ps", bufs=1, space="PSUM")
        s_pool = s_pool_cm.__enter__()
        if True:

            # ---------------- emission helper units --------------------
            def qk_unit(p, which, th):
                w_t, b_t, dst = ((wqT_t, qb_t, qT_t[p]) if which == "q"
                                 else (wkT_t, kb_t, kT_t[p]))
                tsl = slice(512 * th, 512 * (th + 1))
                ps = w_pool.tile([P, 512], f32, name="qk_ps", tag="w_ps",
                                 bufs=4)
                for ec in range(8):
                    nc.tensor.matmul(
                        ps[:], lhsT=w_t[ec][:, P * p:P * (p + 1)],
                        rhs=xT_t[ec][:, tsl],
                        start=(ec == 0), stop=(ec == 7))
                nc.vector.tensor_scalar_add(dst[:, tsl], ps[:],
                                            b_t[:, p:p + 1])

            def v_unit(s):
                ps = w_pool.tile([P, 512], f32, name="v_ps", tag="w_ps",
                                 bufs=4)
                for ec in range(8):
                    nc.tensor.matmul(
                        ps[:], lhsT=xT_t[ec][:, P * s:P * (s + 1)],
                        rhs=wvT_t[ec][:],
                        start=(ec == 0), stop=(ec == 7))
                out = mega[:].rearrange("p (l s c) -> p l s c",
                                        l=8, s=8)[:, :, s:s + 1,
                                                  64:128].squeeze(2)
                nc.vector.tensor_add(
                    out, ps[:].rearrange("p (l c) -> p l c", l=8),
                    vbb_t[:].rearrange("p (l c) -> p l c", l=8))

            denu_ps = {}

            def denu_unit(lh, th):
                # unmasked softmax denominator for relation-masked slots
                p, side = lh // 2, lh % 2
                tsl = slice(512 * th, 512 * (th + 1))
                dps = w_pool.tile([P, 512], f32, name="du_ps", tag="w_ps",
                                  bufs=4)
                denu_ps[(lh, th)] = dps
                for sc in range(8):
                    nc.tensor.matmul(
                        dps[:], lhsT=ones128[:],
                        rhs=e_t[(p, sc)][:, 1024 * side + 512 * th:
                                         1024 * side + 512 * (th + 1)],
                        start=(sc == 0), stop=(sc == 7))
                nc.vector.reciprocal_approx_fast(du_recip[lh][0:64, tsl],
                                                 dps[0:64, :])
                if th == 1:
                    # in-place E *= relM on GpSimd (SBUF-only engine)
                    for sc in range(8):
                        esl = slice(1024 * side, 1024 * side + 1024)
                        nc.gpsimd.tensor_mul(e_t[(p, sc)][:, esl],
                                             e_t[(p, sc)][:, esl],
                                             relM_t[lh][sc][:])

            global_av3_pool = [None]

            def av_unit(p, side, th):
                lh = 2 * p + side
                if p == 3:
                    ps = global_av3_pool[0].tile([P, 512], f32,
                                                 name="av_ps",
                                                 tag="av3_ps", bufs=4)
                else:
                    ps = w_pool.tile([P, 512], f32, name="av_ps",
                                     tag="w_ps", bufs=4)
                av_ps[(lh, th)] = ps
                for sc in range(8):
                    nc.tensor.matmul(
                        ps[:],
                        lhsT=mega[:, 1024 * lh + 128 * sc:
                                  1024 * lh + 128 * (sc + 1)],
                        rhs=e_t[(p, sc)][:, 1024 * side + 512 * th:
                                         1024 * side + 512 * (th + 1)],
                        start=(sc == 0), stop=(sc == 7))

            bc_t = {}

            def cb_pre(p, side, th):
                # per-(side, th-half) reciprocal + re-home, issued right
                # after that av group so the SBUF->SBUF DMA latency hides
                # under the next unit's matmuls
                lh = 2 * p + side
                tsl = slice(512 * th, 512 * (th + 1))
                off = T * side + 512 * th
                if side == 0 and th == 0:
                    bc_t[p] = cb_pool.tile([P, T + T], f32, name="bc",
                                           tag="bc", bufs=2)
                bc = bc_t[p]
                if slot_flags[lh]:
                    nc.sync.dma_start(bc[64:128, off:off + 512],
                                      du_recip[lh][0:64, tsl])
                else:
                    nc.vector.reciprocal_approx_fast(
                        bc[0:64, off:off + 512], av_ps[(lh, th)][0:64, :])
                    nc.sync.dma_start(bc[64:128, off:off + 512],
                                      bc[0:64, off:off + 512])

            def cb_mul(p, side):
                # normalize this side's两 th-halves into oT_sb[p] / tmpb
                lh = 2 * p + side
                bc = bc_t[p]
                if side == 1:
                    for th in range(2):
                        tsl = slice(512 * th, 512 * (th + 1))
                        nc.vector.tensor_mul(oT_sb[p][64:128, tsl],
                                             av_ps[(lh, th)][64:128, :],
                                             bc[64:128, T + 512 * th:
                                                T + 512 * (th + 1)])
                else:
                    tmpb = cb_pool.tile([P, T], bf, name="tmpb",
                                        tag="tmpb", bufs=2)
                    bc_t[(p, "tmpb")] = tmpb
                    for th in range(2):
                        tsl = slice(512 * th, 512 * (th + 1))
                        nc.vector.tensor_mul(tmpb[64:128, tsl],
                                             av_ps[(lh, th)][64:128, :],
                                             bc[64:128, 512 * th:
                                                512 * (th + 1)])
                    nc.sync.dma_start(oT_sb[p][0:64, :], tmpb[64:128, :])

            def cb_unit(p):
                pass

            def oproj_unit(y_pool, tcn):
                yps = y_pool.tile([P, E], f32, name="yps", tag="yps",
                                  bufs=2)
                for eh in range(2):
                    esl = slice(512 * eh, 512 * (eh + 1))
                    for fc in range(4):
                        nc.tensor.matmul(
                            yps[:, esl],
                            lhsT=oT_sb[fc][:, P * tcn:P * (tcn + 1)],
                            rhs=owT_t[fc][:, esl],
                            start=(fc == 0), stop=(fc == 3))
                ysb = ysb_pool.tile([P, E], bf, name="ysb", tag="ysb",
                                    bufs=2)
                nc.scalar.copy(ysb[:], yps[:])
                eng = nc.sync if tcn % 2 == 0 else nc.gpsimd
                eng.dma_start(y_d[P * tcn:P * (tcn + 1), :], ysb[:])

            def sc_step(p, sc):
                st = s_pool.tile([P, 2048], f32, name="s_t", tag="s_ps",
                                 bufs=1)
                csl = slice(P * sc, P * (sc + 1))
                # row-tiled pairs: head A on partitions 0:63, B on 64:127,
                # adjacent emission so the PE runs them concurrently
                nc.tensor.matmul(st[:, 0:512], lhsT=kT_t[p][0:64, csl],
                                 rhs=qT_t[p][0:64, 0:512],
                                 start=True, stop=True)
                nc.tensor.matmul(st[:, 1024:1536], lhsT=kT_t[p][64:128, csl],
                                 rhs=qT_t[p][64:128, 0:512],
                                 start=True, stop=True)
                nc.tensor.matmul(st[:, 512:1024], lhsT=kT_t[p][0:64, csl],
                                 rhs=qT_t[p][0:64, 512:1024],
                                 start=True, stop=True)
                nc.tensor.matmul(st[:, 1536:2048], lhsT=kT_t[p][64:128, csl],
                                 rhs=qT_t[p][64:128, 512:1024],
                                 start=True, stop=True)
                et = e_pool.tile([P, 2048], bf, name="e_t", tag="e_t",
                                 bufs=20)
                e_t[(p, sc)] = et
                nc.scalar.activation(et[:], st[:], Exp)
                if mask_on:
                    for side in range(2):
                        esl = slice(1024 * side, 1024 * side + 1024)
                        nc.gpsimd.tensor_mul(et[:, esl], et[:, esl],
                                             expm_t[sc][:])

            # -------- filler schedule: groups of [PE unit, appendages] --
            def av_groups(pp):
                gs = []
                for side in range(2):
                    for th in range(2):
                        g = [lambda pp=pp, sd=side, th=th:
                             av_unit(pp, sd, th),
                             lambda pp=pp, sd=side, th=th:
                             cb_pre(pp, sd, th)]
                        if th == 1:
                            g.append(lambda pp=pp, sd=side:
                                     cb_mul(pp, sd))
                        gs.append(g)
                return gs

            def pair_fillers(p):
                groups = []
                if p >= 1:
                    for side in range(2):
                        lh = 2 * (p - 1) + side
                        if slot_flags[lh]:
                            groups.append([lambda lh=lh: denu_unit(lh, 0)])
                            groups.append([lambda lh=lh: denu_unit(lh, 1)])
                if p >= 2:
                    groups += av_groups(p - 2)
                if p == 3:
                    groups += av_groups(2)
                if p <= 2:
                    for which in ("q", "k"):
                        for th in range(2):
                            groups.append(
                                [lambda w=which, th=th:
                                 qk_unit(p + 1, w, th)])
                if p == 0:
                    groups += [[lambda s=s: v_unit(s)] for s in range(4)]
                elif p == 1:
                    groups += [[lambda s=s: v_unit(s)] for s in range(4, 8)]
                return groups

            # ---------------- intro ------------------------------------
            wps = s_pool.tile([P, 2048], f32, name="s_t", tag="s_ps",
                              bufs=1)
            for i in range(12):
                nc.tensor.matmul(wps[:, 0:512], lhsT=warm_t[:, 0:128],
                                 rhs=warm_t[:], start=True, stop=True)
            # xT-gated warm matmuls: each becomes ready as its xT chunk
            # lands, keeping the HAM clock warm through the DMA phase
            for ec in range(8):
                for i in range(2):
                    nc.tensor.matmul(wps[:, 0:512], lhsT=warm_t[:, 0:128],
                                     rhs=xT_t[ec][:, 0:512],
                                     start=True, stop=True)
            for which in ("q", "k"):
                for th in range(2):
                    qk_unit(0, which, th)

            # ---------------- main pipelined loop ----------------------
            for p in range(4):
                groups = pair_fillers(p)
                for sc in range(8):
                    if p == 0:
                        sc_step(p, sc)
                    take = -(-len(groups) // (8 - sc))  # ceil
                    for g in groups[:take]:
                        for fn in g:
                            fn()
                    groups = groups[take:]
                    if p > 0:
                        sc_step(p, sc)

            # ---------------- tail: av3 + oproj ----------------------
            # av3 takes the last four w_ps ring slots and interleaves its
            # four (side, th) accumulation groups sc-major, so each matmul
            # becomes ready the moment its exp lands -- the scheduler
            # pulls them into the exp-ladder gaps of the last score phase
            a3_ps = {}
            for side in range(2):
                for th in range(2):
                    a3_ps[(side, th)] = w_pool.tile(
                        [P, 512], f32, name="av_ps", tag="w_ps", bufs=4)
                    av_ps[(6 + side, th)] = a3_ps[(side, th)]
            for sc in range(8):
                for side in range(2):
                    for th in range(2):
                        lh = 6 + side
                        nc.tensor.matmul(
                            a3_ps[(side, th)][:],
                            lhsT=mega[:, 1024 * lh + 128 * sc:
                                      1024 * lh + 128 * (sc + 1)],
                            rhs=e_t[(3, sc)][:, 1024 * side + 512 * th:
                                             1024 * side + 512 * (th + 1)],
                            start=(sc == 0), stop=(sc == 7),
                            skip_group_check=True)
            for side in range(2):
                for th in range(2):
                    cb_pre(3, side, th)
                cb_mul(3, side)
            # y pool takes the score pool's banks the moment the last exp
            # releases them, so each oproj group's fc0-2 matmuls overlap
            # the cb3 normalize chain (only fc3 waits on oT_sb[3])
            s_pool_cm.__exit__(None, None, None)
            y_pool_cm = tc.tile_pool(name="y_ps", bufs=1, space="PSUM")
            y_pool = y_pool_cm.__enter__()
            for tcn in range(8):
                oproj_unit(y_pool, tcn)
            y_pool_cm.__exit__(None, None, None)
            w_pool_cm.__exit__(None, None, None)

    nc.compile()
    return nc


def _get_program(mask_on, slot_flags):
    key = (mask_on, slot_flags)
    if key not in _PROGS:
        _PROGS[key] = _build_program(mask_on, slot_flags)
    return _PROGS[key]


def _prep_inputs(inputs):
    hs = np.asarray(inputs["hidden_states"], dtype=np.float32)
    am = np.asarray(inputs["attention_mask"], dtype=np.float32)
    rel = np.asarray(inputs["relation_inputs"])
    hm = np.asarray(inputs["heads_mask"], dtype=np.float32)
    q_w = np.asarray(inputs["q_w"], dtype=np.float32)
    q_b = np.asarray(inputs["q_b"], dtype=np.float32)
    k_w = np.asarray(inputs["k_w"], dtype=np.float32)
    k_b = np.asarray(inputs["k_b"], dtype=np.float32)
    v_w = np.asarray(inputs["v_w"], dtype=np.float32)
    v_b = np.asarray(inputs["v_b"], dtype=np.float32)
    o_w = np.asarray(inputs["o_w"], dtype=np.float32)
    o_b = np.asarray(inputs["o_b"], dtype=np.float32)

    mask_on = bool(np.any(am != 0.0))
    slot_flags = tuple(
        k == 0 or bool(np.any(hm[[k, 8 + k]] != 0.0)) for k in range(8))

    relbinT = [(rel[b] > 0).T.astype(np.float32) for b in range(B)]
    if mask_on:
        expmT = [np.exp(am[b, 0]).T.astype(BF16) for b in range(B)]

    in_maps = []
    for c in range(N_CORES):
        b, g = c // 2, c % 2
        sl = slice(FH * g, FH * (g + 1))
        im = {
            "xT": np.ascontiguousarray(hs[b].T).astype(BF16),
            "wqT": np.ascontiguousarray((q_w[sl] * SCALING).T).astype(BF16),
            "wkT": np.ascontiguousarray(k_w[sl].T).astype(BF16),
            "wvT": np.ascontiguousarray(v_w[sl].T).astype(BF16),
            "owT": np.ascontiguousarray(o_w[:, sl].T).astype(BF16),
            "qb": np.ascontiguousarray(
                (q_b[sl] * SCALING).reshape(4, P).T).astype(np.float32),
            "kb": np.ascontiguousarray(
                k_b[sl].reshape(4, P).T).astype(np.float32),
            "vbb": np.ascontiguousarray(
                np.broadcast_to(v_b[sl], (P, FH))).astype(np.float32),
        }
        for k in range(8):
            if slot_flags[k]:
                hmv = float(hm[8 * g + k])
                m = (1.0 - hmv) + hmv * relbinT[b]
                im[f"relM{k}"] = m.astype(FP8)
        if mask_on:
            im["expmaskT"] = expmT[b]
        in_maps.append(im)
    return mask_on, slot_flags, in_maps, o_b


def _gather(results, o_b):
    out = np.empty((B, T, E), dtype=np.float32)
    for b in range(B):
        out[b] = (results[2 * b]["y"].astype(np.float32)
                  + results[2 * b + 1]["y"].astype(np.float32) + o_b)
    return out


def run_sharded(inputs, trace=False, trace_kwargs=None):
    from concourse.bass_utils import run_bass_kernel_spmd

    mask_on, slot_flags, in_maps, o_b = _prep_inputs(inputs)
    nc = _get_program(mask_on, slot_flags)
    last_err = None
    for _attempt in range(3):
        try:
            res = run_bass_kernel_spmd(nc, in_maps, list(range(N_CORES)),
                                       trace=trace, **(trace_kwargs or {}))
            return _gather(res.results, o_b), res
        except Exception as e:  # first exec of a fresh NEFF can flake
            last_err = e
    raise last_err


def kernel(**inputs):
    out, _ = run_sharded(inputs)
    return out


# revision 29
# speedup vs baseline: 1.1783x; 1.1783x over previous
"""Trainium2 Bass kernel for BART custom-mask attention.

Problem: B=4, T=S=1024, E=1024, H=16 heads, D=64.
  q = (hs @ q_w.T + q_b) * D**-0.5 ; k/v analogous
  scores = q k^T + attention_mask ; attn = softmax(scores)
  attn(head h) *= (1-hm[h]) + hm[h]*(relation_inputs>0)   (no renorm)
  out = (attn @ v) @ o_w.T + o_b

Sharding: 8 cores = batch (4) x head-group (2, 8 heads each). Each core
computes a 512-feature slice of the attention output and projects it
through the matching o_w columns; the host sums the two half-partials
per batch (plus o_b, folded into the host gather).

Per-core design (bf16 matmuls, fp32 PSUM, fp8 only for the exact 0/1
relation mask):

  - The ScalarE exp stream (32 [128,2048] activations, ~63us busy) and
    the PE matmul stream (~545 matmuls) are co-scheduled: each (pair,
    sc) step issues 4 score matmuls into one [128,2048] PSUM tile, a
    single wide exp, then ~1 "filler" unit of other PE work (qk / v
    projections, denominator matmuls, attn@v of earlier pairs) sized so
    the PE covers the exp drain. Fillers are emitted BEFORE the score
    step for phases>=1 -- the Tile scheduler keeps per-engine emission
    order, so this is what lets them run inside the exp-ladder gaps.
  - Score matmuls are K=64 row-tiled pairs (head A on partitions 0:63,
    head B on 64:127) issued back-to-back; the PE runs each pair
    concurrently in the two row-halves of the array (~3ns apart).
  - exp writes bf16 E tiles [128, 2048] = [eA-th0|eA-th1|eB-th0|eB-th1]
    (fp8 E/V was tried and rejected: attn@v relative error does not
    average down over the contraction, ~3% rel err vs the 2e-2 gate).
  - attn@v: lhsT = [ones(64) | v_h] 128-col blocks from one bf16 mega
    tile; PSUM rows 0:63 get the softmax denominator, 64:127 the data.
    PSUM work tiles are [128,512] th-halves in a 4-deep ring; pair 3's
    four accumulation groups interleave sc-major at the tail so each
    matmul is ready the moment its exp lands.
  - relation-masked slots: unmasked denominator via a ones128 matmul,
    reciprocal stashed to SBUF, then E *= relM (fp8) in place on GpSimd.
  - normalize: reciprocal_approx_fast at partition base 0, SBUF->SBUF
    DMA re-homes to partitions 64:127, one DVE mul per th-half.
  - head: inputs split over both HWDGE queues chunk-by-chunk in qk0
    consumption order; warm-up matmuls (some gated on xT chunk arrival)
    hold the HAM clock at 2.4 GHz through the DMA phase.
  - output projection takes the freed score-pool banks right after the
    last exp; ScalarE (idle after the exp stream) evacuates; y is bf16
    and o_b + the two half-partial sums are folded into the host gather.
"""

import os
import sys

import numpy as np

for _p in ("/opt/trn_rl_repo", "/root/.axon_site/_ro/trn_rl_repo"):
    if os.path.isdir(_p) and _p not in sys.path:
        sys.path.insert(0, _p)
        break

import ml_dtypes

B, T, E, H = 4, 1024, 1024, 16
D = E // H
SCALING = D ** -0.5
N_CORES = 8
FH = 512          # features per core (8 heads x 64)
P = 128
BF16 = ml_dtypes.bfloat16
FP8 = ml_dtypes.float8_e4m3

_PROGS = {}


def _build_program(mask_on, slot_flags):
    import concourse.tile as tile
    from concourse import bacc, mybir
    from contextlib import ExitStack

    bf = mybir.dt.bfloat16
    f32 = mybir.dt.float32
    f8 = mybir.dt.float8e4
    Exp = mybir.ActivationFunctionType.Exp

    nc = bacc.Bacc("TRN2", target_bir_lowering=False, debug=False,
                   num_devices=N_CORES)

    xT_d = nc.declare_dram_parameter("xT", [E, T], bf, isOutput=False)
    wqT_d = nc.declare_dram_parameter("wqT", [E, FH], bf, isOutput=False)
    wkT_d = nc.declare_dram_parameter("wkT", [E, FH], bf, isOutput=False)
    wvT_d = nc.declare_dram_parameter("wvT", [E, FH], bf, isOutput=False)
    owT_d = nc.declare_dram_parameter("owT", [FH, E], bf, isOutput=False)
    qb_d = nc.declare_dram_parameter("qb", [P, 4], f32, isOutput=False)
    kb_d = nc.declare_dram_parameter("kb", [P, 4], f32, isOutput=False)
    vbb_d = nc.declare_dram_parameter("vbb", [P, FH], f32, isOutput=False)
    relM_d = {}
    for k in range(8):
        if slot_flags[k]:
            relM_d[k] = nc.declare_dram_parameter(f"relM{k}", [T, T], f8,
                                                  isOutput=False)
    if mask_on:
        expm_d = nc.declare_dram_parameter("expmaskT", [T, T], bf,
                                           isOutput=False)
    y_d = nc.declare_dram_parameter("y", [T, E], bf, isOutput=True)

    with tile.TileContext(nc) as tc, ExitStack() as ctx:
        persist = ctx.enter_context(tc.tile_pool(name="persist", bufs=1))

        ones128 = persist.tile([P, P], bf, name="ones128", tag="ones128")
        nc.vector.memset(ones128[:], 1.0)
        warm_t = persist.tile([P, 512], bf, name="warm_t", tag="warm_t")
        nc.vector.memset(warm_t[:], 0.5)
        mega = persist.tile([P, 8192], bf, name="vmega", tag="vmega")
        nc.vector.memset(
            mega[:].rearrange("p (g c) -> p g c", c=128)[:, :, 0:64], 1.0)

        # ---------------- input DMA, spread over HWDGE queues ----------
        crit_engines = [nc.sync, nc.scalar]
        late_engines = [nc.gpsimd, nc.sync, nc.scalar, nc.gpsimd]
        dma_rr = [0]

        def dma_in(out_ap, in_ap, late=False):
            engs = late_engines if late else crit_engines
            eng = engs[dma_rr[0] % len(engs)]
            dma_rr[0] += 1
            eng.dma_start(out_ap, in_ap)

        def alloc_tiles(n, rows, cols, nm, dt=bf):
            return [persist.tile([rows, cols], dt, name=f"{nm}{i}",
                                 tag=f"{nm}{i}") for i in range(n)]

        xT_t = alloc_tiles(8, P, T, "xTt")
        wqT_t = alloc_tiles(8, P, FH, "wqTt")
        wkT_t = alloc_tiles(8, P, FH, "wkTt")
        wvT_t = alloc_tiles(8, P, FH, "wvTt")
        owT_t = alloc_tiles(4, P, T, "owTt")
        relM_t = {k: alloc_tiles(8, P, T, f"rMt{k}", dt=f8) for k in relM_d}
        if mask_on:
            expm_t = alloc_tiles(8, P, T, "emt")

        qb_t = persist.tile([P, 4], f32, name="qbt", tag="qbt")
        kb_t = persist.tile([P, 4], f32, name="kbt", tag="kbt")
        vbb_t = persist.tile([P, FH], f32, name="vbbt", tag="vbbt")

        dma_in(qb_t[:], qb_d[:])
        dma_in(kb_t[:], kb_d[:])
        dma_in(vbb_t[:], vbb_d[:])
        # pair-0-critical first: xT + wq/wk chunk by chunk over the two
        # HWDGE queues
        for ec in range(8):
            dma_in(xT_t[ec][:], xT_d[P * ec:P * (ec + 1), :])
            dma_in(wqT_t[ec][:], wqT_d[P * ec:P * (ec + 1), :])
            dma_in(wkT_t[ec][:], wkT_d[P * ec:P * (ec + 1), :])
        for ec in range(8):
            dma_in(wvT_t[ec][:], wvT_d[P * ec:P * (ec + 1), :])
        dma_rr[0] = 0
        for k, dparm in relM_d.items():
            for i in range(8):
                dma_in(relM_t[k][i][:], dparm[P * i:P * (i + 1), :],
                       late=True)
        if mask_on:
            for i in range(8):
                dma_in(expm_t[i][:], expm_d[P * i:P * (i + 1), :],
                       late=True)
        for i in range(4):
            dma_in(owT_t[i][:], owT_d[P * i:P * (i + 1), :], late=True)

        # ---------------- persistent SBUF working tiles ----------------
        qT_t = [persist.tile([P, T], bf, name=f"qTs{p}", tag=f"qTs{p}")
                for p in range(4)]
        kT_t = [persist.tile([P, T], bf, name=f"kTs{p}", tag=f"kTs{p}")
                for p in range(4)]
        # v mega tile: col = lh*1024 + s*128 + c; c in 0:64 -> ones
        # (denominator trick), 64:128 -> v_h for s-chunk s.
        oT_sb = [persist.tile([P, T], bf, name=f"oTs{p}", tag=f"oTs{p}")
                 for p in range(4)]
        du_recip = {}
        for lh in range(8):
            if slot_flags[lh]:
                du_recip[lh] = persist.tile([64, T], f32, name=f"dur{lh}",
                                            tag=f"dur{lh}")

        e_t = {}          # (p, sc) -> fp8 [128, 2048] E tile
        av_ps = {}        # lh -> [128, 1024] f32 PSUM tile

        e_pool = ctx.enter_context(tc.tile_pool(name="e_sb", bufs=1))
        cb_pool = ctx.enter_context(tc.tile_pool(name="cb_sb", bufs=1))
        ysb_pool = ctx.enter_context(tc.tile_pool(name="y_sb", bufs=1))
        w_pool_cm = tc.tile_pool(name="w_ps", bufs=1, space="PSUM")
        w_pool = w_pool_cm.__enter__()
        s_pool_cm = tc.tile_pool(name="s_ps", bufs=1, space="PSUM")
        s_pool = s_pool_cm.__enter__()
        if True:

            # ---------------- emission helper units --------------------
            def qk_unit(p, which, th):
                w_t, b_t, dst = ((wqT_t, qb_t, qT_t[p]) if which == "q"
                                 else (wkT_t, kb_t, kT_t[p]))
                tsl = slice(512 * th, 512 * (th + 1))
                ps = w_pool.tile([P, 512], f32, name="qk_ps", tag="w_ps",
                                 bufs=4)
                for ec in range(8):
                    nc.tensor.matmul(
                        ps[:], lhsT=w_t[ec][:, P * p:P * (p + 1)],
                        rhs=xT_t[ec][:, tsl],
                        start=(ec == 0), stop=(ec == 7))
                nc.vector.tensor_scalar_add(dst[:, tsl], ps[:],
                                            b_t[:, p:p + 1])

            def v_unit(s):
                ps = w_pool.tile([P, 512], f32, name="v_ps", tag="w_ps",
                                 bufs=4)
                for ec in range(8):
                    nc.tensor.matmul(
                        ps[:], lhsT=xT_t[ec][:, P * s:P * (s + 1)],
                        rhs=wvT_t[ec][:],
                        start=(ec == 0), stop=(ec == 7))
                out = mega[:].rearrange("p (l s c) -> p l s c",
                                        l=8, s=8)[:, :, s:s + 1,
                                                  64:128].squeeze(2)
                nc.vector.tensor_add(
                    out, ps[:].rearrange("p (l c) -> p l c", l=8),
                    vbb_t[:].rearrange("p (l c) -> p l c", l=8))

            denu_ps = {}

            def denu_unit(lh, th):
                # unmasked softmax denominator for relation-masked slots
                p, side = lh // 2, lh % 2
                tsl = slice(512 * th, 512 * (th + 1))
                dps = w_pool.tile([P, 512], f32, name="du_ps", tag="w_ps",
                                  bufs=4)
                denu_ps[(lh, th)] = dps
                for sc in range(8):
                    nc.tensor.matmul(
                        dps[:], lhsT=ones128[:],
                        rhs=e_t[(p, sc)][:, 1024 * side + 512 * th:
                                         1024 * side + 512 * (th + 1)],
                        start=(sc == 0), stop=(sc == 7))
                nc.vector.reciprocal_approx_fast(du_recip[lh][0:64, tsl],
                                                 dps[0:64, :])
                if th == 1:
                    # in-place E *= relM on GpSimd (SBUF-only engine)
                    for sc in range(8):
                        esl = slice(1024 * side, 1024 * side + 1024)
                        nc.gpsimd.tensor_mul(e_t[(p, sc)][:, esl],
                                             e_t[(p, sc)][:, esl],
                                             relM_t[lh][sc][:])

            global_av3_pool = [None]

            def av_unit(p, side, th):
                lh = 2 * p + side
                if p == 3:
                    ps = global_av3_pool[0].tile([P, 512], f32,
                                                 name="av_ps",
                                                 tag="av3_ps", bufs=4)
                else:
                    ps = w_pool.tile([P, 512], f32, name="av_ps",
                                     tag="w_ps", bufs=4)
                av_ps[(lh, th)] = ps
                for sc in range(8):
                    nc.tensor.matmul(
                        ps[:],
                        lhsT=mega[:, 1024 * lh + 128 * sc:
                                  1024 * lh + 128 * (sc + 1)],
                        rhs=e_t[(p, sc)][:, 1024 * side + 512 * th:
                                         1024 * side + 512 * (th + 1)],
                        start=(sc == 0), stop=(sc == 7))

            bc_t = {}

            def cb_pre(p, side, th):
                # per-(side, th-half) reciprocal + re-home, issued right
                # after that av group so the SBUF->SBUF DMA latency hides
                # under the next unit's matmuls
                lh = 2 * p + side
                tsl = slice(512 * th, 512 * (th + 1))
                off = T * side + 512 * th
                if side == 0 and th == 0:
                    bc_t[p] = cb_pool.tile([P, T + T], f32, name="bc",
                                           tag="bc", bufs=2)
                bc = bc_t[p]
                if slot_flags[lh]:
                    nc.sync.dma_start(bc[64:128, off:off + 512],
                                      du_recip[lh][0:64, tsl])
                else:
                    nc.vector.reciprocal_approx_fast(
                        bc[0:64, off:off + 512], av_ps[(lh, th)][0:64, :])
                    nc.sync.dma_start(bc[64:128, off:off + 512],
                                      bc[0:64, off:off + 512])

            def cb_mul(p, side):
                # normalize this side's两 th-halves into oT_sb[p] / tmpb
                lh = 2 * p + side
                bc = bc_t[p]
                if side == 1:
                    for th in range(2):
                        tsl = slice(512 * th, 512 * (th + 1))
                        nc.vector.tensor_mul(oT_sb[p][64:128, tsl],
                                             av_ps[(lh, th)][64:128, :],
                                             bc[64:128, T + 512 * th:
                                                T + 512 * (th + 1)])
                else:
                    tmpb = cb_pool.tile([P, T], bf, name="tmpb",
                                        tag="tmpb", bufs=2)
                    bc_t[(p, "tmpb")] = tmpb
                    for th in range(2):
                        tsl = slice(512 * th, 512 * (th + 1))
                        nc.vector.tensor_mul(tmpb[64:128, tsl],
                                             av_ps[(lh, th)][64:128, :],
                                             bc[64:128, 512 * th:
                                                512 * (th + 1)])
                    nc.sync.dma_start(oT_sb[p][0:64, :], tmpb[64:128, :])

            def cb_unit(p):
                pass

            def oproj_unit(y_pool, tcn):
                yps = y_pool.tile([P, E], f32, name="yps", tag="yps",
                                  bufs=2)
                for eh in range(2):
                    esl = slice(512 * eh, 512 * (eh + 1))
                    for fc in range(4):
                        nc.tensor.matmul(
                            yps[:, esl],
                            lhsT=oT_sb[fc][:, P * tcn:P * (tcn + 1)],
                            rhs=owT_t[fc][:, esl],
                            start=(fc == 0), stop=(fc == 3))
                ysb = ysb_pool.tile([P, E], bf, name="ysb", tag="ysb",
                                    bufs=2)
                nc.scalar.copy(ysb[:], yps[:])
                eng = nc.sync if tcn % 2 == 0 else nc.gpsimd
                eng.dma_start(y_d[P * tcn:P * (tcn + 1), :], ysb[:])

            def sc_step(p, sc):
                st = s_pool.tile([P, 2048], f32, name="s_t", tag="s_ps",
                                 bufs=1)
                csl = slice(P * sc, P * (sc + 1))
                # row-tiled pairs: head A on partitions 0:63, B on 64:127,
                # adjacent emission so the PE runs them concurrently
                nc.tensor.matmul(st[:, 0:512], lhsT=kT_t[p][0:64, csl],
                                 rhs=qT_t[p][0:64, 0:512],
                                 start=True, stop=True)
                nc.tensor.matmul(st[:, 1024:1536], lhsT=kT_t[p][64:128, csl],
                                 rhs=qT_t[p][64:128, 0:512],
                                 start=True, stop=True)
                nc.tensor.matmul(st[:, 512:1024], lhsT=kT_t[p][0:64, csl],
                                 rhs=qT_t[p][0:64, 512:1024],
                                 start=True, stop=True)
                nc.tensor.matmul(st[:, 1536:2048], lhsT=kT_t[p][64:128, csl],
                                 rhs=qT_t[p][64:128, 512:1024],
                                 start=True, stop=True)
                et = e_pool.tile([P, 2048], bf, name="e_t", tag="e_t",
                                 bufs=20)
                e_t[(p, sc)] = et
                nc.scalar.activation(et[:], st[:], Exp)
                if mask_on:
                    for side in range(2):
                        esl = slice(1024 * side, 1024 * side + 1024)
                        nc.gpsimd.tensor_mul(et[:, esl], et[:, esl],
                                             expm_t[sc][:])

            # -------- filler schedule: groups of [PE unit, appendages] --
            def av_groups(pp):
                gs = []
                for side in range(2):
                    for th in range(2):
                        g = [lambda pp=pp, sd=side, th=th:
                             av_unit(pp, sd, th),
                             lambda pp=pp, sd=side, th=th:
                             cb_pre(pp, sd, th)]
                        if th == 1:
                            g.append(lambda pp=pp, sd=side:
                                     cb_mul(pp, sd))
                        gs.append(g)
                return gs

            def pair_fillers(p):
                groups = []
                if p >= 1:
                    for side in range(2):
                        lh = 2 * (p - 1) + side
                        if slot_flags[lh]:
                            groups.append([lambda lh=lh: denu_unit(lh, 0)])
                            groups.append([lambda lh=lh: denu_unit(lh, 1)])
                if p >= 2:
                    groups += av_groups(p - 2)
                if p == 3:
                    groups += av_groups(2)
                if p <= 2:
                    for which in ("q", "k"):
                        for th in range(2):
                            groups.append(
                                [lambda w=which, th=th:
                                 qk_unit(p + 1, w, th)])
                if p == 0:
                    groups += [[lambda s=s: v_unit(s)] for s in range(4)]
                elif p == 1:
                    groups += [[lambda s=s: v_unit(s)] for s in range(4, 8)]
                return groups

            # ---------------- intro ------------------------------------
            wps = s_pool.tile([P, 2048], f32, name="s_t", tag="s_ps",
                              bufs=1)
            for i in range(12):
                nc.tensor.matmul(wps[:, 0:512], lhsT=warm_t[:, 0:128],
                                 rhs=warm_t[:], start=True, stop=True)
            # xT-gated warm matmuls: each becomes ready as its xT chunk
            # lands, keeping the HAM clock warm through the DMA phase
            for ec in range(8):
                for i in range(2):
                    nc.tensor.matmul(wps[:, 0:512], lhsT=warm_t[:, 0:128],
                                     rhs=xT_t[ec][:, 0:512],
                                     start=True, stop=True)
            for which in ("q", "k"):
                for th in range(2):
                    qk_unit(0, which, th)

            # ---------------- main pipelined loop ----------------------
            for p in range(4):
                groups = pair_fillers(p)
                for sc in range(8):
                    if p == 0:
                        sc_step(p, sc)
                    take = -(-len(groups) // (8 - sc))  # ceil
                    for g in groups[:take]:
                        for fn in g:
                            fn()
                    groups = groups[take:]
                    if p > 0:
                        sc_step(p, sc)

            # ---------------- tail: av3 + oproj ----------------------
            # av3 takes the last four w_ps ring slots and interleaves its
            # four (side, th) accumulation groups sc-major, so each matmul
            # becomes ready the moment its exp lands -- the scheduler
            # pulls them into the exp-ladder gaps of the last score phase
            a3_ps = {}
            for side in range(2):
                for th in range(2):
                    a3_ps[(side, th)] = w_pool.tile(
                        [P, 512], f32, name="av_ps", tag="w_ps", bufs=4)
                    av_ps[(6 + side, th)] = a3_ps[(side, th)]
            for sc in range(8):
                for side in range(2):
                    for th in range(2):
                        lh = 6 + side
                        nc.tensor.matmul(
                            a3_ps[(side, th)][:],
                            lhsT=mega[:, 1024 * lh + 128 * sc:
                                      1024 * lh + 128 * (sc + 1)],
                            rhs=e_t[(3, sc)][:, 1024 * side + 512 * th:
                                             1024 * side + 512 * (th + 1)],
                            start=(sc == 0), stop=(sc == 7),
                            skip_group_check=True)
            for side in range(2):
                for th in range(2):
                    cb_pre(3, side, th)
                cb_mul(3, side)
            # y pool takes the score pool's banks the moment the last exp
            # releases them, so each oproj group's fc0-2 matmuls overlap
            # the cb3 normalize chain (only fc3 waits on oT_sb[3])
            s_pool_cm.__exit__(None, None, None)
            y_pool_cm = tc.tile_pool(name="y_ps", bufs=1, space="PSUM")
            y_pool = y_pool_cm.__enter__()
            for tcn in range(8):
                oproj_unit(y_pool, tcn)
            y_pool_cm.__exit__(None, None, None)
            w_pool_cm.__exit__(None, None, None)

    nc.compile()
    return nc


def _get_program(mask_on, slot_flags):
    key = (mask_on, slot_flags)
    if key not in _PROGS:
        _PROGS[key] = _build_program(mask_on, slot_flags)
    return _PROGS[key]


def _prep_inputs(inputs):
    hs = np.asarray(inputs["hidden_states"], dtype=np.float32)
    am = np.asarray(inputs["attention_mask"], dtype=np.float32)
    rel = np.asarray(inputs["relation_inputs"])
    hm = np.asarray(inputs["heads_mask"], dtype=np.float32)
    q_w = np.asarray(inputs["q_w"], dtype=np.float32)
    q_b = np.asarray(inputs["q_b"], dtype=np.float32)
    k_w = np.asarray(inputs["k_w"], dtype=np.float32)
    k_b = np.asarray(inputs["k_b"], dtype=np.float32)
    v_w = np.asarray(inputs["v_w"], dtype=np.float32)
    v_b = np.asarray(inputs["v_b"], dtype=np.float32)
    o_w = np.asarray(inputs["o_w"], dtype=np.float32)
    o_b = np.asarray(inputs["o_b"], dtype=np.float32)

    mask_on = bool(np.any(am != 0.0))
    slot_flags = tuple(
        k == 0 or bool(np.any(hm[[k, 8 + k]] != 0.0)) for k in range(8))

    relbinT = [(rel[b] > 0).T.astype(np.float32) for b in range(B)]
    if mask_on:
        expmT = [np.exp(am[b, 0]).T.astype(BF16) for b in range(B)]

    in_maps = []
    for c in range(N_CORES):
        b, g = c // 2, c % 2
        sl = slice(FH * g, FH * (g + 1))
        im = {
            "xT": np.ascontiguousarray(hs[b].T).astype(BF16),
            "wqT": np.ascontiguousarray((q_w[sl] * SCALING).T).astype(BF16),
            "wkT": np.ascontiguousarray(k_w[sl].T).astype(BF16),
            "wvT": np.ascontiguousarray(v_w[sl].T).astype(BF16),
            "owT": np.ascontiguousarray(o_w[:, sl].T).astype(BF16),
            "qb": np.ascontiguousarray(
                (q_b[sl] * SCALING).reshape(4, P).T).astype(np.float32),
            "kb": np.ascontiguousarray(
                k_b[sl].reshape(4, P).T).astype(np.float32),
            "vbb": np.ascontiguousarray(
                np.broadcast_to(v_b[sl], (P, FH))).astype(np.float32),
        }
        for k in range(8):
            if slot_flags[k]:
                hmv = float(hm[8 * g + k])
                m = (1.0 - hmv) + hmv * relbinT[b]
                im[f"relM{k}"] = m.astype(FP8)
        if mask_on:
            im["expmaskT"] = expmT[b]
        in_maps.append(im)
    return mask_on, slot_flags, in_maps, o_b


def _gather(results, o_b):
    out = np.empty((B, T, E), dtype=np.float32)
    for b in range(B):
        out[b] = (results[2 * b]["y"].astype(np.float32)
                  + results[2 * b + 1]["y"].astype(np.float32) + o_b)
    return out


def run_sharded(inputs, trace=False, trace_kwargs=None):
    from concourse.bass_utils import run_bass_kernel_spmd

    mask_on, slot_flags, in_maps, o_b = _prep_inputs(inputs)
    nc = _get_program(mask_on, slot_flags)
    last_err = None
    for _attempt in range(3):
        try:
            res = run_bass_kernel_spmd(nc, in_maps, list(range(N_CORES)),
                                       trace=trace, **(trace_kwargs or {}))
            return _gather(res.results, o_b), res
        except Exception as e:  # first exec of a fresh NEFF can flake
            last_err = e
    raise last_err


def kernel(**inputs):
    out, _ = run_sharded(inputs)
    return out


# revision 34
# speedup vs baseline: 1.2357x; 1.0488x over previous
"""Trainium2 Bass kernel for BART custom-mask attention.

Problem: B=4, T=S=1024, E=1024, H=16 heads, D=64.
  q = (hs @ q_w.T + q_b) * D**-0.5 ; k/v analogous
  scores = q k^T + attention_mask ; attn = softmax(scores)
  attn(head h) *= (1-hm[h]) + hm[h]*(relation_inputs>0)   (no renorm)
  out = (attn @ v) @ o_w.T + o_b

Sharding: 8 cores = batch (4) x head-group (2, 8 heads each). Each core
computes a 512-feature slice of the attention output and projects it
through the matching o_w columns; the host sums the two half-partials
per batch (plus o_b, folded into the host gather).

Per-core design (bf16 matmuls, fp32 PSUM, fp8 only for the exact 0/1
relation mask):

  - The ScalarE exp stream (32 [128,2048] activations, ~63us busy) and
    the PE matmul stream (~545 matmuls) are co-scheduled: each (pair,
    sc) step issues 4 score matmuls into one [128,2048] PSUM tile, a
    single wide exp, then ~1 "filler" unit of other PE work (qk / v
    projections, denominator matmuls, attn@v of earlier pairs) sized so
    the PE covers the exp drain. Fillers are emitted BEFORE the score
    step for phases>=1 -- the Tile scheduler keeps per-engine emission
    order, so this is what lets them run inside the exp-ladder gaps.
  - Score matmuls are K=64 row-tiled pairs (head A on partitions 0:63,
    head B on 64:127) issued back-to-back; the PE runs each pair
    concurrently in the two row-halves of the array (~3ns apart).
  - exp writes bf16 E tiles [128, 2048] = [eA-th0|eA-th1|eB-th0|eB-th1]
    (fp8 E/V was tried and rejected: attn@v relative error does not
    average down over the contraction, ~3% rel err vs the 2e-2 gate).
  - attn@v: lhsT = [ones(64) | v_h] 128-col blocks from one bf16 mega
    tile; PSUM rows 0:63 get the softmax denominator, 64:127 the data.
    PSUM work tiles are [128,512] th-halves in a 4-deep ring; pair 3's
    four accumulation groups interleave sc-major at the tail so each
    matmul is ready the moment its exp lands.
  - relation-masked slots: unmasked denominator via a ones128 matmul,
    reciprocal stashed to SBUF, then E *= relM (fp8) in place on GpSimd.
  - normalize: reciprocal_approx_fast at partition base 0, SBUF->SBUF
    DMA re-homes to partitions 64:127, one DVE mul per th-half.
  - head: inputs split over both HWDGE queues chunk-by-chunk in qk0
    consumption order; warm-up matmuls (some gated on xT chunk arrival)
    hold the HAM clock at 2.4 GHz through the DMA phase.
  - output projection takes the freed score-pool banks right after the
    last exp; ScalarE (idle after the exp stream) evacuates; y is bf16
    and o_b + the two half-partial sums are folded into the host gather.
"""

import os
import sys

import numpy as np

for _p in ("/opt/trn_rl_repo", "/root/.axon_site/_ro/trn_rl_repo"):
    if os.path.isdir(_p) and _p not in sys.path:
        sys.path.insert(0, _p)
        break

import ml_dtypes

B, T, E, H = 4, 1024, 1024, 16
D = E // H
SCALING = D ** -0.5
N_CORES = 8
FH = 512          # features per core (8 heads x 64)
P = 128
BF16 = ml_dtypes.bfloat16
FP8 = ml_dtypes.float8_e4m3

_PROGS = {}


def _build_program(mask_on, slot_flags):
    import concourse.tile as tile
    from concourse import bacc, mybir
    from contextlib import ExitStack

    bf = mybir.dt.bfloat16
    f32 = mybir.dt.float32
    f8 = mybir.dt.float8e4
    Exp = mybir.ActivationFunctionType.Exp

    nc = bacc.Bacc("TRN2", target_bir_lowering=False, debug=False,
                   num_devices=N_CORES)

    xT_d = nc.declare_dram_parameter("xT", [E, T], bf, isOutput=False)
    wqT_d = nc.declare_dram_parameter("wqT", [E, FH], bf, isOutput=False)
    wkT_d = nc.declare_dram_parameter("wkT", [E, FH], bf, isOutput=False)
    wvT_d = nc.declare_dram_parameter("wvT", [E, FH], bf, isOutput=False)
    owT_d = nc.declare_dram_parameter("owT", [FH, E], bf, isOutput=False)
    qb_d = nc.declare_dram_parameter("qb", [P, 4], f32, isOutput=False)
    kb_d = nc.declare_dram_parameter("kb", [P, 4], f32, isOutput=False)
    vbb_d = nc.declare_dram_parameter("vbb", [P, FH], f32, isOutput=False)
    relM_d = {}
    for k in range(8):
        if slot_flags[k]:
            relM_d[k] = nc.declare_dram_parameter(f"relM{k}", [T, T], f8,
                                                  isOutput=False)
    if mask_on:
        expm_d = nc.declare_dram_parameter("expmaskT", [T, T], bf,
                                           isOutput=False)
    y_d = nc.declare_dram_parameter("y", [T, E], bf, isOutput=True)

    with tile.TileContext(nc) as tc, ExitStack() as ctx:
        persist = ctx.enter_context(tc.tile_pool(name="persist", bufs=1))

        ones128 = persist.tile([P, P], bf, name="ones128", tag="ones128")
        nc.vector.memset(ones128[:], 1.0)
        warm_t = persist.tile([P, 512], bf, name="warm_t", tag="warm_t")
        nc.vector.memset(warm_t[:], 0.5)
        mega = persist.tile([P, 8192], bf, name="vmega", tag="vmega")
        nc.vector.memset(
            mega[:].rearrange("p (g c) -> p g c", c=128)[:, :, 0:64], 1.0)

        # ---------------- input DMA, spread over HWDGE queues ----------
        crit_engines = [nc.sync, nc.scalar]
        late_engines = [nc.sync, nc.scalar]
        dma_rr = [0]

        def dma_in(out_ap, in_ap, late=False):
            engs = late_engines if late else crit_engines
            eng = engs[dma_rr[0] % len(engs)]
            dma_rr[0] += 1
            eng.dma_start(out_ap, in_ap)

        def alloc_tiles(n, rows, cols, nm, dt=bf):
            return [persist.tile([rows, cols], dt, name=f"{nm}{i}",
                                 tag=f"{nm}{i}") for i in range(n)]

        xT_t = alloc_tiles(8, P, T, "xTt")
        wqT_t = alloc_tiles(8, P, FH, "wqTt")
        wkT_t = alloc_tiles(8, P, FH, "wkTt")
        wvT_t = alloc_tiles(8, P, FH, "wvTt")
        owT_t = alloc_tiles(4, P, T, "owTt")
        relM_t = {k: alloc_tiles(8, P, T, f"rMt{k}", dt=f8) for k in relM_d}
        if mask_on:
            expm_t = alloc_tiles(8, P, T, "emt")

        qb_t = persist.tile([P, 4], f32, name="qbt", tag="qbt")
        kb_t = persist.tile([P, 4], f32, name="kbt", tag="kbt")
        vbb_t = persist.tile([P, FH], f32, name="vbbt", tag="vbbt")

        dma_in(qb_t[:], qb_d[:])
        dma_in(kb_t[:], kb_d[:])
        dma_in(vbb_t[:], vbb_d[:])
        # pair-0-critical first: xT + wq/wk chunk by chunk over the two
        # HWDGE queues
        for ec in range(8):
            dma_in(xT_t[ec][:], xT_d[P * ec:P * (ec + 1), :])
            dma_in(wqT_t[ec][:], wqT_d[P * ec:P * (ec + 1), :])
            dma_in(wkT_t[ec][:], wkT_d[P * ec:P * (ec + 1), :])
        for ec in range(8):
            dma_in(wvT_t[ec][:], wvT_d[P * ec:P * (ec + 1), :])
        dma_rr[0] = 0
        for k, dparm in relM_d.items():
            for i in range(8):
                dma_in(relM_t[k][i][:], dparm[P * i:P * (i + 1), :],
                       late=True)
        if mask_on:
            for i in range(8):
                dma_in(expm_t[i][:], expm_d[P * i:P * (i + 1), :],
                       late=True)
        for i in range(4):
            dma_in(owT_t[i][:], owT_d[P * i:P * (i + 1), :], late=True)

        # ---------------- persistent SBUF working tiles ----------------
        qT_t = [persist.tile([P, T], bf, name=f"qTs{p}", tag=f"qTs{p}")
                for p in range(4)]
        kT_t = [persist.tile([P, T], bf, name=f"kTs{p}", tag=f"kTs{p}")
                for p in range(4)]
        # v mega tile: col = lh*1024 + s*128 + c; c in 0:64 -> ones
        # (denominator trick), 64:128 -> v_h for s-chunk s.
        oT_sb = [persist.tile([P, T], bf, name=f"oTs{p}", tag=f"oTs{p}")
                 for p in range(4)]
        du_recip = {}
        for lh in range(8):
            if slot_flags[lh]:
                du_recip[lh] = persist.tile([64, T], f32, name=f"dur{lh}",
                                            tag=f"dur{lh}")

        e_t = {}          # (p, sc) -> fp8 [128, 2048] E tile
        av_ps = {}        # lh -> [128, 1024] f32 PSUM tile

        e_pool = ctx.enter_context(tc.tile_pool(name="e_sb", bufs=1))
        cb_pool = ctx.enter_context(tc.tile_pool(name="cb_sb", bufs=1))
        ysb_pool = ctx.enter_context(tc.tile_pool(name="y_sb", bufs=1))
        w_pool_cm = tc.tile_pool(name="w_ps", bufs=1, space="PSUM")
        w_pool = w_pool_cm.__enter__()
        s_pool_cm = tc.tile_pool(name="s_ps", bufs=1, space="PSUM")
        s_pool = s_pool_cm.__enter__()
        if True:

            # ---------------- emission helper units --------------------
            def qk_unit(p, which, th):
                w_t, b_t, dst = ((wqT_t, qb_t, qT_t[p]) if which == "q"
                                 else (wkT_t, kb_t, kT_t[p]))
                tsl = slice(512 * th, 512 * (th + 1))
                ps = w_pool.tile([P, 512], f32, name="qk_ps", tag="w_ps",
                                 bufs=4)
                for ec in range(8):
                    nc.tensor.matmul(
                        ps[:], lhsT=w_t[ec][:, P * p:P * (p + 1)],
                        rhs=xT_t[ec][:, tsl],
                        start=(ec == 0), stop=(ec == 7))
                nc.vector.tensor_scalar_add(dst[:, tsl], ps[:],
                                            b_t[:, p:p + 1])

            def v_unit(s):
                ps = w_pool.tile([P, 512], f32, name="v_ps", tag="w_ps",
                                 bufs=4)
                for ec in range(8):
                    nc.tensor.matmul(
                        ps[:], lhsT=xT_t[ec][:, P * s:P * (s + 1)],
                        rhs=wvT_t[ec][:],
                        start=(ec == 0), stop=(ec == 7))
                out = mega[:].rearrange("p (l s c) -> p l s c",
                                        l=8, s=8)[:, :, s:s + 1,
                                                  64:128].squeeze(2)
                nc.vector.tensor_add(
                    out, ps[:].rearrange("p (l c) -> p l c", l=8),
                    vbb_t[:].rearrange("p (l c) -> p l c", l=8))

            denu_ps = {}

            def denu_unit(lh, th):
                # unmasked softmax denominator for relation-masked slots
                p, side = lh // 2, lh % 2
                tsl = slice(512 * th, 512 * (th + 1))
                dps = w_pool.tile([P, 512], f32, name="du_ps", tag="w_ps",
                                  bufs=4)
                denu_ps[(lh, th)] = dps
                for sc in range(8):
                    nc.tensor.matmul(
                        dps[:], lhsT=ones128[:],
                        rhs=e_t[(p, sc)][:, 1024 * side + 512 * th:
                                         1024 * side + 512 * (th + 1)],
                        start=(sc == 0), stop=(sc == 7))
                nc.vector.reciprocal_approx_fast(du_recip[lh][0:64, tsl],
                                                 dps[0:64, :])
                if th == 1:
                    # in-place E *= relM on GpSimd (SBUF-only engine)
                    for sc in range(8):
                        esl = slice(1024 * side, 1024 * side + 1024)
                        nc.gpsimd.tensor_mul(e_t[(p, sc)][:, esl],
                                             e_t[(p, sc)][:, esl],
                                             relM_t[lh][sc][:])

            global_av3_pool = [None]

            def av_unit(p, side, th):
                lh = 2 * p + side
                if p == 3:
                    ps = global_av3_pool[0].tile([P, 512], f32,
                                                 name="av_ps",
                                                 tag="av3_ps", bufs=4)
                else:
                    ps = w_pool.tile([P, 512], f32, name="av_ps",
                                     tag="w_ps", bufs=4)
                av_ps[(lh, th)] = ps
                for sc in range(8):
                    nc.tensor.matmul(
                        ps[:],
                        lhsT=mega[:, 1024 * lh + 128 * sc:
                                  1024 * lh + 128 * (sc + 1)],
                        rhs=e_t[(p, sc)][:, 1024 * side + 512 * th:
                                         1024 * side + 512 * (th + 1)],
                        start=(sc == 0), stop=(sc == 7))

            bc_t = {}

            def cb_pre(p, side, th):
                # per-(side, th-half) reciprocal + re-home, issued right
                # after that av group so the SBUF->SBUF DMA latency hides
                # under the next unit's matmuls
                lh = 2 * p + side
                tsl = slice(512 * th, 512 * (th + 1))
                off = T * side + 512 * th
                if side == 0 and th == 0:
                    bc_t[p] = cb_pool.tile([P, T + T], f32, name="bc",
                                           tag="bc", bufs=2)
                bc = bc_t[p]
                if slot_flags[lh]:
                    nc.sync.dma_start(bc[64:128, off:off + 512],
                                      du_recip[lh][0:64, tsl])
                else:
                    nc.vector.reciprocal_approx_fast(
                        bc[0:64, off:off + 512], av_ps[(lh, th)][0:64, :])
                    nc.sync.dma_start(bc[64:128, off:off + 512],
                                      bc[0:64, off:off + 512])

            def cb_mul(p, side):
                # normalize this side's两 th-halves into oT_sb[p] / tmpb
                lh = 2 * p + side
                bc = bc_t[p]
                if side == 1:
                    for th in range(2):
                        tsl = slice(512 * th, 512 * (th + 1))
                        nc.vector.tensor_mul(oT_sb[p][64:128, tsl],
                                             av_ps[(lh, th)][64:128, :],
                                             bc[64:128, T + 512 * th:
                                                T + 512 * (th + 1)])
                else:
                    tmpb = cb_pool.tile([P, T], bf, name="tmpb",
                                        tag="tmpb", bufs=2)
                    bc_t[(p, "tmpb")] = tmpb
                    for th in range(2):
                        tsl = slice(512 * th, 512 * (th + 1))
                        nc.vector.tensor_mul(tmpb[64:128, tsl],
                                             av_ps[(lh, th)][64:128, :],
                                             bc[64:128, 512 * th:
                                                512 * (th + 1)])
                    nc.sync.dma_start(oT_sb[p][0:64, :], tmpb[64:128, :])

            def cb_unit(p):
                pass

            def oproj_unit(y_pool, tcn):
                yps = y_pool.tile([P, E], f32, name="yps", tag="yps",
                                  bufs=2)
                for eh in range(2):
                    esl = slice(512 * eh, 512 * (eh + 1))
                    for fc in range(4):
                        nc.tensor.matmul(
                            yps[:, esl],
                            lhsT=oT_sb[fc][:, P * tcn:P * (tcn + 1)],
                            rhs=owT_t[fc][:, esl],
                            start=(fc == 0), stop=(fc == 3))
                ysb = ysb_pool.tile([P, E], bf, name="ysb", tag="ysb",
                                    bufs=2)
                nc.scalar.copy(ysb[:], yps[:])
                nc.sync.dma_start(y_d[P * tcn:P * (tcn + 1), :], ysb[:])

            def sc_step(p, sc):
                st = s_pool.tile([P, 2048], f32, name="s_t", tag="s_ps",
                                 bufs=1)
                csl = slice(P * sc, P * (sc + 1))
                # row-tiled pairs: head A on partitions 0:63, B on 64:127,
                # adjacent emission so the PE runs them concurrently
                nc.tensor.matmul(st[:, 0:512], lhsT=kT_t[p][0:64, csl],
                                 rhs=qT_t[p][0:64, 0:512],
                                 start=True, stop=True)
                nc.tensor.matmul(st[:, 1024:1536], lhsT=kT_t[p][64:128, csl],
                                 rhs=qT_t[p][64:128, 0:512],
                                 start=True, stop=True)
                nc.tensor.matmul(st[:, 512:1024], lhsT=kT_t[p][0:64, csl],
                                 rhs=qT_t[p][0:64, 512:1024],
                                 start=True, stop=True)
                nc.tensor.matmul(st[:, 1536:2048], lhsT=kT_t[p][64:128, csl],
                                 rhs=qT_t[p][64:128, 512:1024],
                                 start=True, stop=True)
                et = e_pool.tile([P, 2048], bf, name="e_t", tag="e_t",
                                 bufs=20)
                e_t[(p, sc)] = et
                nc.scalar.activation(et[:], st[:], Exp)
                if mask_on:
                    for side in range(2):
                        esl = slice(1024 * side, 1024 * side + 1024)
                        nc.gpsimd.tensor_mul(et[:, esl], et[:, esl],
                                             expm_t[sc][:])

            # -------- filler schedule: groups of [PE unit, appendages] --
            def av_groups(pp):
                gs = []
                for side in range(2):
                    for th in range(2):
                        g = [lambda pp=pp, sd=side, th=th:
                             av_unit(pp, sd, th),
                             lambda pp=pp, sd=side, th=th:
                             cb_pre(pp, sd, th)]
                        if th == 1:
                            g.append(lambda pp=pp, sd=side:
                                     cb_mul(pp, sd))
                        gs.append(g)
                return gs

            def pair_fillers(p):
                groups = []
                if p >= 1:
                    for side in range(2):
                        lh = 2 * (p - 1) + side
                        if slot_flags[lh]:
                            groups.append([lambda lh=lh: denu_unit(lh, 0)])
                            groups.append([lambda lh=lh: denu_unit(lh, 1)])
                if p >= 2:
                    groups += av_groups(p - 2)
                if p == 3:
                    groups += av_groups(2)
                if p <= 2:
                    for which in ("q", "k"):
                        for th in range(2):
                            groups.append(
                                [lambda w=which, th=th:
                                 qk_unit(p + 1, w, th)])
                if p == 0:
                    groups += [[lambda s=s: v_unit(s)] for s in range(4)]
                elif p == 1:
                    groups += [[lambda s=s: v_unit(s)] for s in range(4, 8)]
                return groups

            # ---------------- intro ------------------------------------
            wps = s_pool.tile([P, 2048], f32, name="s_t", tag="s_ps",
                              bufs=1)
            for i in range(12):
                nc.tensor.matmul(wps[:, 0:512], lhsT=warm_t[:, 0:128],
                                 rhs=warm_t[:], start=True, stop=True)
            # xT-gated warm matmuls: each becomes ready as its xT chunk
            # lands, keeping the HAM clock warm through the DMA phase
            for ec in range(8):
                for i in range(2):
                    nc.tensor.matmul(wps[:, 0:512], lhsT=warm_t[:, 0:128],
                                     rhs=xT_t[ec][:, 0:512],
                                     start=True, stop=True)
            for which in ("q", "k"):
                for th in range(2):
                    qk_unit(0, which, th)

            # ---------------- main pipelined loop ----------------------
            for p in range(4):
                groups = pair_fillers(p)
                for sc in range(8):
                    if p == 0:
                        sc_step(p, sc)
                    take = -(-len(groups) // (8 - sc))  # ceil
                    for g in groups[:take]:
                        for fn in g:
                            fn()
                    groups = groups[take:]
                    if p > 0:
                        sc_step(p, sc)

            # ---------------- tail: av3 + oproj ----------------------
            # av3 takes the last four w_ps ring slots and interleaves its
            # four (side, th) accumulation groups sc-major, so each matmul
            # becomes ready the moment its exp lands -- the scheduler
            # pulls them into the exp-ladder gaps of the last score phase
            a3_ps = {}
            for side in range(2):
                for th in range(2):
                    a3_ps[(side, th)] = w_pool.tile(
                        [P, 512], f32, name="av_ps", tag="w_ps", bufs=4)
                    av_ps[(6 + side, th)] = a3_ps[(side, th)]
                    # ring-gated warm-up: fills the exp-ladder gaps and
                    # holds the HAM clock; the real group's start=True
                    # resets the bank
                    for _ in range(2):
                        nc.tensor.matmul(a3_ps[(side, th)][:],
                                         lhsT=warm_t[:, 0:128],
                                         rhs=warm_t[:],
                                         start=True, stop=True)
            for sc in range(8):
                for side in range(2):
                    for th in range(2):
                        lh = 6 + side
                        nc.tensor.matmul(
                            a3_ps[(side, th)][:],
                            lhsT=mega[:, 1024 * lh + 128 * sc:
                                      1024 * lh + 128 * (sc + 1)],
                            rhs=e_t[(3, sc)][:, 1024 * side + 512 * th:
                                             1024 * side + 512 * (th + 1)],
                            start=(sc == 0), stop=(sc == 7),
                            skip_group_check=True)
            for side in range(2):
                for th in range(2):
                    cb_pre(3, side, th)
                cb_mul(3, side)
            for _ in range(6):
                nc.tensor.matmul(a3_ps[(0, 0)][:], lhsT=warm_t[:, 0:128],
                                 rhs=warm_t[:], start=True, stop=True)
            # y pool takes the score pool's banks the moment the last exp
            # releases them, so each oproj group's fc0-2 matmuls overlap
            # the cb3 normalize chain (only fc3 waits on oT_sb[3])
            s_pool_cm.__exit__(None, None, None)
            y_pool_cm = tc.tile_pool(name="y_ps", bufs=1, space="PSUM")
            y_pool = y_pool_cm.__enter__()
            for tcn in range(8):
                oproj_unit(y_pool, tcn)
            y_pool_cm.__exit__(None, None, None)
            w_pool_cm.__exit__(None, None, None)

    nc.compile()
    return nc


def _get_program(mask_on, slot_flags):
    key = (mask_on, slot_flags)
    if key not in _PROGS:
        _PROGS[key] = _build_program(mask_on, slot_flags)
    return _PROGS[key]


def _prep_inputs(inputs):
    hs = np.asarray(inputs["hidden_states"], dtype=np.float32)
    am = np.asarray(inputs["attention_mask"], dtype=np.float32)
    rel = np.asarray(inputs["relation_inputs"])
    hm = np.asarray(inputs["heads_mask"], dtype=np.float32)
    q_w = np.asarray(inputs["q_w"], dtype=np.float32)
    q_b = np.asarray(inputs["q_b"], dtype=np.float32)
    k_w = np.asarray(inputs["k_w"], dtype=np.float32)
    k_b = np.asarray(inputs["k_b"], dtype=np.float32)
    v_w = np.asarray(inputs["v_w"], dtype=np.float32)
    v_b = np.asarray(inputs["v_b"], dtype=np.float32)
    o_w = np.asarray(inputs["o_w"], dtype=np.float32)
    o_b = np.asarray(inputs["o_b"], dtype=np.float32)

    mask_on = bool(np.any(am != 0.0))
    slot_flags = tuple(
        k == 0 or bool(np.any(hm[[k, 8 + k]] != 0.0)) for k in range(8))

    relbinT = [(rel[b] > 0).T.astype(np.float32) for b in range(B)]
    if mask_on:
        expmT = [np.exp(am[b, 0]).T.astype(BF16) for b in range(B)]

    in_maps = []
    for c in range(N_CORES):
        b, g = c // 2, c % 2
        sl = slice(FH * g, FH * (g + 1))
        im = {
            "xT": np.ascontiguousarray(hs[b].T).astype(BF16),
            "wqT": np.ascontiguousarray((q_w[sl] * SCALING).T).astype(BF16),
            "wkT": np.ascontiguousarray(k_w[sl].T).astype(BF16),
            "wvT": np.ascontiguousarray(v_w[sl].T).astype(BF16),
            "owT": np.ascontiguousarray(o_w[:, sl].T).astype(BF16),
            "qb": np.ascontiguousarray(
                (q_b[sl] * SCALING).reshape(4, P).T).astype(np.float32),
            "kb": np.ascontiguousarray(
                k_b[sl].reshape(4, P).T).astype(np.float32),
            "vbb": np.ascontiguousarray(
                np.broadcast_to(v_b[sl], (P, FH))).astype(np.float32),
        }
        for k in range(8):
            if slot_flags[k]:
                hmv = float(hm[8 * g + k])
                m = (1.0 - hmv) + hmv * relbinT[b]
                im[f"relM{k}"] = m.astype(FP8)
        if mask_on:
            im["expmaskT"] = expmT[b]
        in_maps.append(im)
    return mask_on, slot_flags, in_maps, o_b


def _gather(results, o_b):
    out = np.empty((B, T, E), dtype=np.float32)
    for b in range(B):
        out[b] = (results[2 * b]["y"].astype(np.float32)
                  + results[2 * b + 1]["y"].astype(np.float32) + o_b)
    return out


def run_sharded(inputs, trace=False, trace_kwargs=None):
    from concourse.bass_utils import run_bass_kernel_spmd

    mask_on, slot_flags, in_maps, o_b = _prep_inputs(inputs)
    nc = _get_program(mask_on, slot_flags)
    last_err = None
    for _attempt in range(3):
        try:
            res = run_bass_kernel_spmd(nc, in_maps, list(range(N_CORES)),
                                       trace=trace, **(trace_kwargs or {}))
            return _gather(res.results, o_b), res
        except Exception as e:  # first exec of a fresh NEFF can flake
            last_err = e
    raise last_err


def kernel(**inputs):
    out, _ = run_sharded(inputs)
    return out
